# revision 29
# baseline (speedup 1.0000x reference)
"""Trainium2 Bass kernel for nn_Attention_12146167513140.

Distributed dense attention over 8 NeuronCores.

Sharding: core c in 0..7 -> (b = c//4, head-pair hp = c%4).  Each core
computes the full [3072 q x 3072 k] attention for its 2 heads of its
batch, producing a partial output projection [3072, 256]; the host sums
the 4 partials per batch and adds b_out.

Device pipeline per core (all matmuls bf16, accumulation f32 in PSUM):
  A) kv = s2 @ Wkv_pair -> rms-norm k -> kT tiles (PE transpose),
     v (+ones col) -> vx tiles
  B) q = s1e @ Wq_pair -> rms-norm q -> qT tiles
  C) flash-style: scoresT[k,q] = kT.T @ qT (33rd contraction row carries
     the additive mask as a rank-1 term), exp on ScalarE (scale fused),
     PV: oT[h] += vx.T @ expT (ones column accumulates the softmax
     denominator Z), normalize by 1/Z
  D) out_partial = oT.T @ Wout_pair

Host-side prep: sinusoidal positional embedding (index arithmetic),
transposes, bf16 casts, mask row encoding.
"""

import numpy as np
import ml_dtypes

import concourse.bacc as bacc
import concourse.mybir as mybir
from concourse import bass_utils
from concourse.tile import TileContext
from concourse.alu_op_type import AluOpType
from concourse.mybir import ActivationFunctionType as AF

AX = mybir.AxisListType
BF = mybir.dt.bfloat16
F32 = mybir.dt.float32
bf16 = ml_dtypes.bfloat16

B, N1, N2 = 2, 3072, 3072
C_S, H, D = 256, 8, 32
INF = 100000.0
EPS = 1e-8
SCALE = float(np.sqrt(1.0 / (3 * D)))

NCORES = 8
HPC = 2            # heads per core
KCH = N2 // 128    # 24 key chunks
QCH = N1 // 128    # 24 q row chunks
QB = 512           # q block for scores free dim
NQB = N1 // QB     # 6
VW = D + 1         # 33: v columns + ones column for Z

_cache = {}


def _build(use_g2: bool):
    nc = bacc.Bacc("TRN2", target_bir_lowering=False, debug=False, num_devices=NCORES)

    s1T_d = nc.dram_tensor("s1T", [C_S, N1], BF, kind="ExternalInput")
    s2T_d = nc.dram_tensor("s2T", [C_S, N2], BF, kind="ExternalInput")
    wq_d = nc.dram_tensor("wq", [C_S, HPC * D], BF, kind="ExternalInput")
    wkv_d = nc.dram_tensor("wkv", [C_S, HPC * 2 * D], BF, kind="ExternalInput")
    wout_d = nc.dram_tensor("wout", [HPC * D, C_S], BF, kind="ExternalInput")
    qm_d = nc.dram_tensor("qm", [1, N1], BF, kind="ExternalInput")
    km_d = nc.dram_tensor("km", [1, N2], BF, kind="ExternalInput")
    id_d = nc.dram_tensor("ident", [128, 128], BF, kind="ExternalInput")
    z_d = nc.dram_tensor("zeros", [96, N2], BF, kind="ExternalInput")
    if use_g2:
        g2_d = nc.dram_tensor("g2", [128, HPC * D], BF, kind="ExternalInput")
    zdr = [nc.dram_tensor(f"zscratch{h}", [1, N1], F32, kind="Internal")
           for h in range(HPC)]
    out_d = nc.dram_tensor("out", [N1, C_S], F32, kind="ExternalOutput")

    with TileContext(nc) as tc:
        with (
            tc.tile_pool(name="const", bufs=1) as cpool,
            tc.tile_pool(name="stage", bufs=1) as stage,
            tc.tile_pool(name="work", bufs=3) as work,
            tc.tile_pool(name="norm", bufs=3) as npool,
            tc.tile_pool(name="expp", bufs=3) as expp,
            tc.tile_pool(name="epi", bufs=4) as epi,
            tc.tile_pool(name="psA", bufs=2, space="PSUM") as psA,
            tc.tile_pool(name="psO", bufs=4, space="PSUM") as psO,
        ):
            # ---- constants / staging ----
            ident = cpool.tile([128, 128], BF)
            nc.sync.dma_start(ident[:, :], id_d.ap())
            epst = cpool.tile([128, 1], F32)
            nc.vector.memset(epst[:, :], EPS)

            wq_sb = cpool.tile([128, HPC * D], BF, tag="wq")
            wq_sb2 = cpool.tile([128, HPC * D], BF, tag="wq2")
            nc.sync.dma_start(wq_sb[:, :], wq_d.ap()[0:128, :])
            nc.sync.dma_start(wq_sb2[:, :], wq_d.ap()[128:256, :])
            wkv_sb = cpool.tile([128, HPC * 2 * D], BF, tag="wkv")
            wkv_sb2 = cpool.tile([128, HPC * 2 * D], BF, tag="wkv2")
            nc.sync.dma_start(wkv_sb[:, :], wkv_d.ap()[0:128, :])
            nc.sync.dma_start(wkv_sb2[:, :], wkv_d.ap()[128:256, :])
            wout_sb = cpool.tile([HPC * D, C_S], BF, tag="wout")
            nc.sync.dma_start(wout_sb[:, :], wout_d.ap())
            if use_g2:
                g2_sb = cpool.tile([128, HPC * D], BF, tag="g2")
                nc.sync.dma_start(g2_sb[:, :], g2_d.ap())

            s1T = [cpool.tile([128, N1], BF, tag=f"s1T{i}", name=f"s1T{i}") for i in range(2)]
            s2T = [cpool.tile([128, N2], BF, tag=f"s2T{i}", name=f"s2T{i}") for i in range(2)]
            for i in range(2):
                for j in range(4):
                    sl = slice(j * (N1 // 4), (j + 1) * (N1 // 4))
                    nc.sync.dma_start(s1T[i][:, sl], s1T_d.ap()[i * 128:(i + 1) * 128, sl])
                    nc.sync.dma_start(s2T[i][:, sl], s2T_d.ap()[i * 128:(i + 1) * 128, sl])

            # per-head transposed tensors; row 32 carries the mask row.
            # Padded to 128 partitions (rows 33..127 zero) so the QK matmul
            # streams full-width K=128 (K=33 runs ~1.7x slower).
            kT = [cpool.tile([128, N2], BF, tag=f"kT{h}", name=f"kT{h}") for h in range(HPC)]
            qT = [cpool.tile([128, N1], BF, tag=f"qT{h}", name=f"qT{h}") for h in range(HPC)]
            for h in range(HPC):
                nc.sync.dma_start(kT[h][32:128, :], z_d.ap())
                nc.sync.dma_start(qT[h][32:128, :], z_d.ap())
                nc.sync.dma_start(kT[h][32:33, :], km_d.ap())
                nc.sync.dma_start(qT[h][32:33, :], qm_d.ap())

            # v-extended: per kchunk, per head: [v(32) | ones(1)] columns
            vx = cpool.tile([128, KCH * HPC * VW], BF, tag="vx")
            nc.gpsimd.memset(
                vx[:, :].rearrange("p (n w) -> p n w", w=VW)[:, :, 32:33], 1.0
            )

            oT_sb = cpool.tile([HPC * D, N1], BF, tag="oT")
            zrow = [cpool.tile([1, N1], F32, tag=f"zrow{h}", name=f"zrow{h}")
                    for h in range(HPC)]
            zp = [cpool.tile([128, QCH], F32, tag=f"zp{h}", name=f"zp{h}")
                  for h in range(HPC)]
            rzp = [cpool.tile([128, QCH], F32, tag=f"rzp{h}", name=f"rzp{h}")
                   for h in range(HPC)]

            # ---- phases A/B: projections + rms-norm + transposes ----
            # Two passes with ONE batched Sqrt (avoids ACT table thrash
            # between the Sqrt and Exp function sets) and ONE batched
            # reciprocal (DVE reciprocal has a large per-op floor).
            NCHUNK = KCH + QCH  # 48
            kcp_all = cpool.tile([128, NCHUNK * HPC * D], F32, tag="kcp_all")
            ss_all = cpool.tile([128, NCHUNK * HPC], F32, tag="ss_all")

            def pass1(ci, kc, sT, w1, w2, vdst):
                ncol = w1.shape[1]
                pp = psO.tile([128, ncol], F32, tag="oT", name=f"pp{ci}")
                nc.tensor.matmul(pp[:, :], sT[0][:, kc * 128:(kc + 1) * 128], w1[:, :],
                                 start=True, stop=False)
                nc.tensor.matmul(pp[:, :], sT[1][:, kc * 128:(kc + 1) * 128], w2[:, :],
                                 start=False, stop=True)
                per_h = ncol // HPC
                kcp = kcp_all[:, ci * HPC * D:(ci + 1) * HPC * D]
                nc.vector.tensor_copy(
                    kcp.rearrange("p (h d) -> p h d", d=D),
                    pp[:, :].rearrange("p (h x) -> p h x", h=HPC)[:, :, 0:D])
                sq = npool.tile([128, HPC * D], F32, tag="sq", name=f"sq{ci}")
                nc.gpsimd.tensor_tensor(sq[:, :], kcp, kcp, AluOpType.mult)
                nc.vector.reduce_sum(
                    ss_all[:, ci * HPC:(ci + 1) * HPC],
                    sq[:, :].rearrange("p (h d) -> p h d", d=D), axis=AX.X)
                if vdst is not None:  # kv: copy v columns into vx (+cast bf16)
                    nc.vector.tensor_copy(
                        vdst[:, kc * HPC * VW:(kc + 1) * HPC * VW]
                        .rearrange("p (h w) -> p h w", w=VW)[:, :, 0:D],
                        pp[:, :].rearrange("p (h x) -> p h x", h=HPC)[:, :, D:2 * D])

            sr_all = cpool.tile([128, NCHUNK * HPC], F32, tag="sr_all")
            rinv_all = cpool.tile([128, NCHUNK * HPC], F32, tag="rinv_all")

            def pass2(ci, kc, dstT, qside):
                kcp = kcp_all[:, ci * HPC * D:(ci + 1) * HPC * D]
                pre = npool.tile([128, HPC * D], BF, tag="pre", name=f"pre{ci}")
                for h in range(HPC):
                    nc.gpsimd.tensor_scalar(
                        pre[:, h * D:(h + 1) * D], kcp[:, h * D:(h + 1) * D],
                        rinv_all[:, ci * HPC + h:ci * HPC + h + 1], None,
                        AluOpType.mult)
                if use_g2 and qside:  # q side carries the gq*gk factor
                    nc.vector.tensor_tensor(pre[:, :], pre[:, :], g2_sb[:, :],
                                            AluOpType.mult)
                tp = psO.tile([HPC * D, 128], BF, tag="oT", name=f"tp{ci}")
                nc.tensor.transpose(tp[:, :], pre[:, :], ident[:, :])
                for h in range(HPC):
                    nc.vector.tensor_copy(
                        dstT[h][0:D, kc * 128:(kc + 1) * 128],
                        tp[h * D:(h + 1) * D, :])

            def norm_batch(chunks, kvside):
                """pass1 for a contiguous chunk batch, one batched sqrt+recip,
                then pass2.  Sub-batching keeps the prologue pipelined: phase C
                unblocks as soon as the early q/kv batches land."""
                for kc in chunks:
                    if kvside:
                        pass1(kc, kc, s2T, wkv_sb, wkv_sb2, vx)
                    else:
                        pass1(KCH + kc, kc, s1T, wq_sb, wq_sb2, None)
                ci0 = (chunks[0] if kvside else KCH + chunks[0]) * HPC
                ci1 = (chunks[-1] if kvside else KCH + chunks[-1]) * HPC + HPC
                sl = slice(ci0, ci1)
                nc.scalar.activation(sr_all[:, sl], ss_all[:, sl], AF.Sqrt,
                                     bias=epst[:, :], scale=1.0 / D)
                nc.vector.reciprocal(rinv_all[:, sl], sr_all[:, sl])
                for kc in chunks:
                    if kvside:
                        pass2(kc, kc, kT, False)
                    else:
                        pass2(KCH + kc, kc, qT, True)

            def attend(qb):
                qsl = slice(qb * QB, (qb + 1) * QB)
                oT = [psO.tile([VW, QB], F32, tag="oT", name=f"oT_{qb}_{i}")
                      for i in range(HPC)]
                for kc in range(KCH):
                    sc = psA.tile([128, HPC * QB], F32, tag="sc",
                                  name=f"sc_{qb}_{kc}")
                    for h in range(HPC):
                        nc.tensor.matmul(
                            sc[:, h * QB:(h + 1) * QB],
                            kT[h][:, kc * 128:(kc + 1) * 128],
                            qT[h][:, qsl],
                            start=True, stop=True)
                    ex = expp.tile([128, HPC * QB], BF, tag="ex",
                                   name=f"ex_{qb}_{kc}")
                    nc.scalar.activation(ex[:, :], sc[:, :], AF.Exp, scale=SCALE)
                    for h in range(HPC):
                        nc.tensor.matmul(
                            oT[h][:, :],
                            vx[:, (kc * HPC + h) * VW:(kc * HPC + h + 1) * VW],
                            ex[:, h * QB:(h + 1) * QB],
                            start=(kc == 0), stop=(kc == KCH - 1))
                for h in range(HPC):
                    # unnormalized o -> sbuf bf16; Z row -> per-head Z vector
                    nc.vector.tensor_copy(oT_sb[h * D:(h + 1) * D, qsl],
                                          oT[h][0:D, :])
                    nc.vector.tensor_copy(zrow[h][0:1, qsl], oT[h][32:33, :])
                # 1/Z in q-partition-major layout: rzp[h][p,qc] = 1/Z_h[qc*128+p]
                # via a DRAM bounce (partition<->free transpose), so the
                # reciprocal runs on [128, 4] instead of [1, 512].
                for h in range(HPC):
                    nc.sync.dma_start(zdr[h].ap()[0:1, qsl], zrow[h][0:1, qsl])
                    nc.sync.dma_start(
                        zp[h][:, qb * 4:(qb + 1) * 4],
                        zdr[h].ap()[0:1, qsl].rearrange("o (c p) -> o p c", p=128)[0])
                    nc.vector.reciprocal(rzp[h][:, qb * 4:(qb + 1) * 4],
                                         zp[h][:, qb * 4:(qb + 1) * 4])

            def proj_out(qc):
                osl = slice(qc * 128, (qc + 1) * 128)
                op0 = psO.tile([128, C_S], F32, tag="oT", name=f"op0_{qc}")
                nc.tensor.matmul(op0[:, :], oT_sb[0:D, osl], wout_sb[0:D, :],
                                 start=True, stop=True)
                op1 = psO.tile([128, C_S], F32, tag="oT", name=f"op1_{qc}")
                nc.tensor.matmul(op1[:, :], oT_sb[D:2 * D, osl],
                                 wout_sb[D:2 * D, :], start=True, stop=True)
                t0 = work.tile([128, C_S], F32, tag="t0", name=f"t0_{qc}")
                nc.vector.tensor_scalar(t0[:, :], op0[:, :],
                                        rzp[0][:, qc:qc + 1], None,
                                        AluOpType.mult)
                ops = work.tile([128, C_S], F32, tag="osb", name=f"osb_{qc}")
                nc.vector.scalar_tensor_tensor(
                    ops[:, :], op1[:, :], rzp[1][:, qc:qc + 1], t0[:, :],
                    AluOpType.mult, AluOpType.add)
                nc.sync.dma_start(out_d.ap()[osl, :], ops[:, :])

            # prologue schedule: attend(0) needs qT chunks 0-3 and kT
            # incrementally; emit the minimum before it and overlap the rest.
            # The out-projection for q-block qb is emitted after attend(qb+1)
            # so its matmuls fill PE bubbles instead of forming a tail.
            norm_batch(list(range(0, 8)), kvside=False)
            norm_batch(list(range(0, 12)), kvside=True)
            norm_batch(list(range(12, 24)), kvside=True)
            norm_batch(list(range(8, 24)), kvside=False)
            for qb in range(NQB):
                attend(qb)
            for qc in range(QCH):
                proj_out(qc)

    nc.compile()
    return nc


def _host_prep(inputs):
    s1 = np.asarray(inputs["s1"], np.float32)
    s2 = np.asarray(inputs["s2"], np.float32)
    ridx1 = np.asarray(inputs["ridx1"], np.int32)
    ct1 = np.asarray(inputs["ct1"], np.int32)
    mask1 = np.asarray(inputs["mask1"], np.int32)
    mask2 = np.asarray(inputs["mask2"], np.int32)
    Wq = np.asarray(inputs["Wq"], np.float32)
    Wkv = np.asarray(inputs["Wkv"], np.float32)
    Wout = np.asarray(inputs["Wout"], np.float32)
    gq = np.asarray(inputs["gq"], np.float32)
    gk = np.asarray(inputs["gk"], np.float32)

    ct_idx = np.take_along_axis(ridx1, ct1[:, None], axis=1)
    pos = (ridx1 - ct_idx).astype(np.float32)
    half = C_S // 2
    freqs = np.exp(-np.log(10000.0) * np.arange(half, dtype=np.float32) / half)
    ang = pos[..., None] * freqs
    s1e = s1 + np.concatenate([np.sin(ang), np.cos(ang)], axis=-1).astype(np.float32)

    m1 = mask1.astype(np.float32)
    km = (mask2.astype(np.float32) - 1.0) * INF / SCALE

    g2 = gq * gk
    use_g2 = not np.allclose(g2, 1.0)

    ident = np.eye(128, dtype=bf16)
    zeros = np.zeros((96, N2), dtype=bf16)
    in_maps = []
    for c in range(NCORES):
        b, hp = c // 4, c % 4
        m = {
            "s1T": np.ascontiguousarray(s1e[b].T).astype(bf16),
            "s2T": np.ascontiguousarray(s2[b].T).astype(bf16),
            "wq": np.ascontiguousarray(Wq[:, hp * HPC * D:(hp + 1) * HPC * D]).astype(bf16),
            "wkv": np.ascontiguousarray(Wkv[:, hp * HPC * 2 * D:(hp + 1) * HPC * 2 * D]).astype(bf16),
            "wout": np.ascontiguousarray(Wout[hp * HPC * D:(hp + 1) * HPC * D, :]).astype(bf16),
            "qm": m1[b][None, :].astype(bf16),
            "km": km[b][None, :].astype(bf16),
            "ident": ident,
            "zeros": zeros,
        }
        if use_g2:
            m["g2"] = np.tile(g2[None, hp * HPC * D:(hp + 1) * HPC * D], (128, 1)).astype(bf16)
        in_maps.append(m)
    return in_maps, use_g2, np.asarray(inputs["b_out"], np.float32)


def _run(inputs, trace=False, **kw):
    in_maps, use_g2, b_out = _host_prep(inputs)
    key = ("nc", use_g2)
    if key not in _cache:
        _cache[key] = _build(use_g2)
    nc = _cache[key]
    res = bass_utils.run_bass_kernel_spmd(
        nc, in_maps, core_ids=list(range(NCORES)), trace=trace, **kw)
    out = np.zeros((B, N1, C_S), np.float32)
    for c in range(NCORES):
        out[c // 4] += res.results[c]["out"]
    out += b_out[None, None, :]
    return out, res


def kernel(**inputs) -> np.ndarray:
    out, _ = _run(inputs, trace=False)
    return out


# revision 30
# speedup vs baseline: 1.1228x; 1.1228x over previous
"""Trainium2 Bass kernel for nn_Attention_12146167513140.

Distributed dense attention over 8 NeuronCores.

Sharding: core c in 0..7 -> (b = c//4, head-pair hp = c%4).  Each core
computes the full [3072 q x 3072 k] attention for its 2 heads of its
batch, producing a partial output projection [3072, 256]; the host sums
the 4 partials per batch and adds b_out.

Device pipeline per core (all matmuls bf16, accumulation f32 in PSUM):
  A) kv = s2 @ Wkv_pair -> rms-norm k -> kT tiles (PE transpose),
     v (+ones col) -> vx tiles
  B) q = s1e @ Wq_pair -> rms-norm q -> qT tiles
  C) flash-style: scoresT[k,q] = kT.T @ qT (33rd contraction row carries
     the additive mask as a rank-1 term), exp on ScalarE (scale fused),
     PV: oT[h] += vx.T @ expT (ones column accumulates the softmax
     denominator Z), normalize by 1/Z
  D) out_partial = oT.T @ Wout_pair

Host-side prep: sinusoidal positional embedding (index arithmetic),
transposes, bf16 casts, mask row encoding.
"""

import numpy as np
import ml_dtypes

import concourse.bacc as bacc
import concourse.mybir as mybir
from concourse import bass_utils
from concourse.tile import TileContext
from concourse.alu_op_type import AluOpType
from concourse.mybir import ActivationFunctionType as AF

AX = mybir.AxisListType
BF = mybir.dt.bfloat16
F32 = mybir.dt.float32
bf16 = ml_dtypes.bfloat16

B, N1, N2 = 2, 3072, 3072
C_S, H, D = 256, 8, 32
INF = 100000.0
EPS = 1e-8
SCALE = float(np.sqrt(1.0 / (3 * D)))

NCORES = 8
HPC = 2            # heads per core
KCH = N2 // 128    # 24 key chunks
QCH = N1 // 128    # 24 q row chunks
QB = 512           # q block for scores free dim
NQB = N1 // QB     # 6
VW = D + 1         # 33: v columns + ones column for Z

_cache = {}


def _build(use_g2: bool):
    nc = bacc.Bacc("TRN2", target_bir_lowering=False, debug=False, num_devices=NCORES)

    s1T_d = nc.dram_tensor("s1T", [C_S, N1], BF, kind="ExternalInput")
    s2T_d = nc.dram_tensor("s2T", [C_S, N2], BF, kind="ExternalInput")
    wq_d = nc.dram_tensor("wq", [C_S, HPC * D], BF, kind="ExternalInput")
    wkv_d = nc.dram_tensor("wkv", [C_S, HPC * 2 * D], BF, kind="ExternalInput")
    wout_d = nc.dram_tensor("wout", [HPC * D, C_S], BF, kind="ExternalInput")
    qm_d = nc.dram_tensor("qm", [1, N1], BF, kind="ExternalInput")
    km_d = nc.dram_tensor("km", [1, N2], BF, kind="ExternalInput")
    id_d = nc.dram_tensor("ident", [128, 128], BF, kind="ExternalInput")
    z_d = nc.dram_tensor("zeros", [96, N2], BF, kind="ExternalInput")
    if use_g2:
        g2_d = nc.dram_tensor("g2", [128, HPC * D], BF, kind="ExternalInput")
    zdr = [nc.dram_tensor(f"zscratch{h}", [1, N1], F32, kind="Internal")
           for h in range(HPC)]
    out_d = nc.dram_tensor("out", [N1, C_S], F32, kind="ExternalOutput")

    with TileContext(nc) as tc:
        with (
            tc.tile_pool(name="const", bufs=1) as cpool,
            tc.tile_pool(name="stage", bufs=1) as stage,
            tc.tile_pool(name="work", bufs=3) as work,
            tc.tile_pool(name="norm", bufs=3) as npool,
            tc.tile_pool(name="expp", bufs=3) as expp,
            tc.tile_pool(name="epi", bufs=4) as epi,
            tc.tile_pool(name="psA", bufs=2, space="PSUM") as psA,
            tc.tile_pool(name="psO", bufs=4, space="PSUM") as psO,
        ):
            # ---- constants / staging ----
            ident = cpool.tile([128, 128], BF)
            nc.sync.dma_start(ident[:, :], id_d.ap())
            epst = cpool.tile([128, 1], F32)
            nc.vector.memset(epst[:, :], EPS)

            wq_sb = cpool.tile([128, HPC * D], BF, tag="wq")
            wq_sb2 = cpool.tile([128, HPC * D], BF, tag="wq2")
            nc.sync.dma_start(wq_sb[:, :], wq_d.ap()[0:128, :])
            nc.sync.dma_start(wq_sb2[:, :], wq_d.ap()[128:256, :])
            wkv_sb = cpool.tile([128, HPC * 2 * D], BF, tag="wkv")
            wkv_sb2 = cpool.tile([128, HPC * 2 * D], BF, tag="wkv2")
            nc.sync.dma_start(wkv_sb[:, :], wkv_d.ap()[0:128, :])
            nc.sync.dma_start(wkv_sb2[:, :], wkv_d.ap()[128:256, :])
            wout_sb = cpool.tile([HPC * D, C_S], BF, tag="wout")
            nc.sync.dma_start(wout_sb[:, :], wout_d.ap())
            if use_g2:
                g2_sb = cpool.tile([128, HPC * D], BF, tag="g2")
                nc.sync.dma_start(g2_sb[:, :], g2_d.ap())

            s1T = [cpool.tile([128, N1], BF, tag=f"s1T{i}", name=f"s1T{i}") for i in range(2)]
            s2T = [cpool.tile([128, N2], BF, tag=f"s2T{i}", name=f"s2T{i}") for i in range(2)]
            for i in range(2):
                for j in range(4):
                    sl = slice(j * (N1 // 4), (j + 1) * (N1 // 4))
                    nc.sync.dma_start(s1T[i][:, sl], s1T_d.ap()[i * 128:(i + 1) * 128, sl])
                    nc.sync.dma_start(s2T[i][:, sl], s2T_d.ap()[i * 128:(i + 1) * 128, sl])

            # per-head transposed tensors; row 32 carries the mask row.
            # Padded to 128 partitions (rows 33..127 zero) so the QK matmul
            # streams full-width K=128 (K=33 runs ~1.7x slower).
            kT = [cpool.tile([128, N2], BF, tag=f"kT{h}", name=f"kT{h}") for h in range(HPC)]
            qT = [cpool.tile([128, N1], BF, tag=f"qT{h}", name=f"qT{h}") for h in range(HPC)]
            for h in range(HPC):
                nc.sync.dma_start(kT[h][32:128, :], z_d.ap())
                nc.sync.dma_start(qT[h][32:128, :], z_d.ap())
                nc.sync.dma_start(kT[h][32:33, :], km_d.ap())
                nc.sync.dma_start(qT[h][32:33, :], qm_d.ap())

            # v-extended: per kchunk, per head: [v(32) | ones(1)] columns
            vx = cpool.tile([128, KCH * HPC * VW], BF, tag="vx")
            nc.gpsimd.memset(
                vx[:, :].rearrange("p (n w) -> p n w", w=VW)[:, :, 32:33], 1.0
            )

            oT_sb = cpool.tile([HPC * D, N1], BF, tag="oT")
            zrow = [cpool.tile([1, N1], F32, tag=f"zrow{h}", name=f"zrow{h}")
                    for h in range(HPC)]
            zp = [cpool.tile([128, QCH], F32, tag=f"zp{h}", name=f"zp{h}")
                  for h in range(HPC)]
            rzp = [cpool.tile([128, QCH], F32, tag=f"rzp{h}", name=f"rzp{h}")
                   for h in range(HPC)]

            # ---- phases A/B: projections + rms-norm + transposes ----
            # Two passes with ONE batched Sqrt (avoids ACT table thrash
            # between the Sqrt and Exp function sets) and ONE batched
            # reciprocal (DVE reciprocal has a large per-op floor).
            NCHUNK = KCH + QCH  # 48
            kcp_all = cpool.tile([128, NCHUNK * HPC * D], F32, tag="kcp_all")
            ss_all = cpool.tile([128, NCHUNK * HPC], F32, tag="ss_all")

            def pass1(ci, kc, sT, w1, w2, vdst):
                ncol = w1.shape[1]
                pp = psO.tile([128, ncol], F32, tag="oT", name=f"pp{ci}")
                nc.tensor.matmul(pp[:, :], sT[0][:, kc * 128:(kc + 1) * 128], w1[:, :],
                                 start=True, stop=False)
                nc.tensor.matmul(pp[:, :], sT[1][:, kc * 128:(kc + 1) * 128], w2[:, :],
                                 start=False, stop=True)
                per_h = ncol // HPC
                kcp = kcp_all[:, ci * HPC * D:(ci + 1) * HPC * D]
                nc.vector.tensor_copy(
                    kcp.rearrange("p (h d) -> p h d", d=D),
                    pp[:, :].rearrange("p (h x) -> p h x", h=HPC)[:, :, 0:D])
                sq = npool.tile([128, HPC * D], F32, tag="sq", name=f"sq{ci}")
                nc.vector.tensor_tensor(sq[:, :], kcp, kcp, AluOpType.mult)
                nc.vector.reduce_sum(
                    ss_all[:, ci * HPC:(ci + 1) * HPC],
                    sq[:, :].rearrange("p (h d) -> p h d", d=D), axis=AX.X)
                if vdst is not None:  # kv: copy v columns into vx (+cast bf16)
                    nc.vector.tensor_copy(
                        vdst[:, kc * HPC * VW:(kc + 1) * HPC * VW]
                        .rearrange("p (h w) -> p h w", w=VW)[:, :, 0:D],
                        pp[:, :].rearrange("p (h x) -> p h x", h=HPC)[:, :, D:2 * D])

            sr_all = cpool.tile([128, NCHUNK * HPC], F32, tag="sr_all")
            rinv_all = cpool.tile([128, NCHUNK * HPC], F32, tag="rinv_all")

            def pass2(ci, kc, dstT, qside):
                kcp = kcp_all[:, ci * HPC * D:(ci + 1) * HPC * D]
                pre = npool.tile([128, HPC * D], BF, tag="pre", name=f"pre{ci}")
                for h in range(HPC):
                    nc.vector.tensor_scalar(
                        pre[:, h * D:(h + 1) * D], kcp[:, h * D:(h + 1) * D],
                        rinv_all[:, ci * HPC + h:ci * HPC + h + 1], None,
                        AluOpType.mult)
                if use_g2 and qside:  # q side carries the gq*gk factor
                    nc.vector.tensor_tensor(pre[:, :], pre[:, :], g2_sb[:, :],
                                            AluOpType.mult)
                tp = psO.tile([HPC * D, 128], BF, tag="oT", name=f"tp{ci}")
                nc.tensor.transpose(tp[:, :], pre[:, :], ident[:, :])
                for h in range(HPC):
                    nc.vector.tensor_copy(
                        dstT[h][0:D, kc * 128:(kc + 1) * 128],
                        tp[h * D:(h + 1) * D, :])

            def norm_batch(chunks, kvside):
                """pass1 for a contiguous chunk batch, one batched sqrt+recip,
                then pass2.  Sub-batching keeps the prologue pipelined: phase C
                unblocks as soon as the early q/kv batches land."""
                for kc in chunks:
                    if kvside:
                        pass1(kc, kc, s2T, wkv_sb, wkv_sb2, vx)
                    else:
                        pass1(KCH + kc, kc, s1T, wq_sb, wq_sb2, None)
                ci0 = (chunks[0] if kvside else KCH + chunks[0]) * HPC
                ci1 = (chunks[-1] if kvside else KCH + chunks[-1]) * HPC + HPC
                sl = slice(ci0, ci1)
                nc.scalar.activation(sr_all[:, sl], ss_all[:, sl], AF.Sqrt,
                                     bias=epst[:, :], scale=1.0 / D)
                nc.vector.reciprocal(rinv_all[:, sl], sr_all[:, sl])
                for kc in chunks:
                    if kvside:
                        pass2(kc, kc, kT, False)
                    else:
                        pass2(KCH + kc, kc, qT, True)

            def attend(qb):
                qsl = slice(qb * QB, (qb + 1) * QB)
                oT = [psO.tile([VW, QB], F32, tag="oT", name=f"oT_{qb}_{i}")
                      for i in range(HPC)]
                for kc in range(KCH):
                    sc = psA.tile([128, HPC * QB], F32, tag="sc",
                                  name=f"sc_{qb}_{kc}")
                    for h in range(HPC):
                        nc.tensor.matmul(
                            sc[:, h * QB:(h + 1) * QB],
                            kT[h][:, kc * 128:(kc + 1) * 128],
                            qT[h][:, qsl],
                            start=True, stop=True)
                    ex = expp.tile([128, HPC * QB], BF, tag="ex",
                                   name=f"ex_{qb}_{kc}")
                    nc.scalar.activation(ex[:, :], sc[:, :], AF.Exp, scale=SCALE)
                    for h in range(HPC):
                        nc.tensor.matmul(
                            oT[h][:, :],
                            vx[:, (kc * HPC + h) * VW:(kc * HPC + h + 1) * VW],
                            ex[:, h * QB:(h + 1) * QB],
                            start=(kc == 0), stop=(kc == KCH - 1))
                for h in range(HPC):
                    # unnormalized o -> sbuf bf16; Z row -> per-head Z vector
                    nc.vector.tensor_copy(oT_sb[h * D:(h + 1) * D, qsl],
                                          oT[h][0:D, :])
                    nc.vector.tensor_copy(zrow[h][0:1, qsl], oT[h][32:33, :])
                # 1/Z in q-partition-major layout: rzp[h][p,qc] = 1/Z_h[qc*128+p]
                # via a DRAM bounce (partition<->free transpose), so the
                # reciprocal runs on [128, 4] instead of [1, 512].
                for h in range(HPC):
                    nc.sync.dma_start(zdr[h].ap()[0:1, qsl], zrow[h][0:1, qsl])
                    nc.sync.dma_start(
                        zp[h][:, qb * 4:(qb + 1) * 4],
                        zdr[h].ap()[0:1, qsl].rearrange("o (c p) -> o p c", p=128)[0])
                    nc.vector.reciprocal(rzp[h][:, qb * 4:(qb + 1) * 4],
                                         zp[h][:, qb * 4:(qb + 1) * 4])

            def proj_out(qc):
                osl = slice(qc * 128, (qc + 1) * 128)
                op0 = psO.tile([128, C_S], F32, tag="oT", name=f"op0_{qc}")
                nc.tensor.matmul(op0[:, :], oT_sb[0:D, osl], wout_sb[0:D, :],
                                 start=True, stop=True)
                op1 = psO.tile([128, C_S], F32, tag="oT", name=f"op1_{qc}")
                nc.tensor.matmul(op1[:, :], oT_sb[D:2 * D, osl],
                                 wout_sb[D:2 * D, :], start=True, stop=True)
                t0 = work.tile([128, C_S], F32, tag="t0", name=f"t0_{qc}")
                nc.vector.tensor_scalar(t0[:, :], op0[:, :],
                                        rzp[0][:, qc:qc + 1], None,
                                        AluOpType.mult)
                ops = work.tile([128, C_S], F32, tag="osb", name=f"osb_{qc}")
                nc.vector.scalar_tensor_tensor(
                    ops[:, :], op1[:, :], rzp[1][:, qc:qc + 1], t0[:, :],
                    AluOpType.mult, AluOpType.add)
                nc.sync.dma_start(out_d.ap()[osl, :], ops[:, :])

            # prologue schedule: attend(0) needs qT chunks 0-3 and kT
            # incrementally; emit the minimum before it and overlap the rest.
            # The out-projection for q-block qb is emitted after attend(qb+1)
            # so its matmuls fill PE bubbles instead of forming a tail.
            norm_batch(list(range(0, 8)), kvside=False)
            norm_batch(list(range(0, 12)), kvside=True)
            norm_batch(list(range(12, 24)), kvside=True)
            norm_batch(list(range(8, 24)), kvside=False)
            for qb in range(NQB):
                attend(qb)
            for qc in range(QCH):
                proj_out(qc)

    nc.compile()
    return nc


def _host_prep(inputs):
    s1 = np.asarray(inputs["s1"], np.float32)
    s2 = np.asarray(inputs["s2"], np.float32)
    ridx1 = np.asarray(inputs["ridx1"], np.int32)
    ct1 = np.asarray(inputs["ct1"], np.int32)
    mask1 = np.asarray(inputs["mask1"], np.int32)
    mask2 = np.asarray(inputs["mask2"], np.int32)
    Wq = np.asarray(inputs["Wq"], np.float32)
    Wkv = np.asarray(inputs["Wkv"], np.float32)
    Wout = np.asarray(inputs["Wout"], np.float32)
    gq = np.asarray(inputs["gq"], np.float32)
    gk = np.asarray(inputs["gk"], np.float32)

    ct_idx = np.take_along_axis(ridx1, ct1[:, None], axis=1)
    pos = (ridx1 - ct_idx).astype(np.float32)
    half = C_S // 2
    freqs = np.exp(-np.log(10000.0) * np.arange(half, dtype=np.float32) / half)
    ang = pos[..., None] * freqs
    s1e = s1 + np.concatenate([np.sin(ang), np.cos(ang)], axis=-1).astype(np.float32)

    m1 = mask1.astype(np.float32)
    km = (mask2.astype(np.float32) - 1.0) * INF / SCALE

    g2 = gq * gk
    use_g2 = not np.allclose(g2, 1.0)

    ident = np.eye(128, dtype=bf16)
    zeros = np.zeros((96, N2), dtype=bf16)
    in_maps = []
    for c in range(NCORES):
        b, hp = c // 4, c % 4
        m = {
            "s1T": np.ascontiguousarray(s1e[b].T).astype(bf16),
            "s2T": np.ascontiguousarray(s2[b].T).astype(bf16),
            "wq": np.ascontiguousarray(Wq[:, hp * HPC * D:(hp + 1) * HPC * D]).astype(bf16),
            "wkv": np.ascontiguousarray(Wkv[:, hp * HPC * 2 * D:(hp + 1) * HPC * 2 * D]).astype(bf16),
            "wout": np.ascontiguousarray(Wout[hp * HPC * D:(hp + 1) * HPC * D, :]).astype(bf16),
            "qm": m1[b][None, :].astype(bf16),
            "km": km[b][None, :].astype(bf16),
            "ident": ident,
            "zeros": zeros,
        }
        if use_g2:
            m["g2"] = np.tile(g2[None, hp * HPC * D:(hp + 1) * HPC * D], (128, 1)).astype(bf16)
        in_maps.append(m)
    return in_maps, use_g2, np.asarray(inputs["b_out"], np.float32)


def _run(inputs, trace=False, **kw):
    in_maps, use_g2, b_out = _host_prep(inputs)
    key = ("nc", use_g2)
    if key not in _cache:
        _cache[key] = _build(use_g2)
    nc = _cache[key]
    res = bass_utils.run_bass_kernel_spmd(
        nc, in_maps, core_ids=list(range(NCORES)), trace=trace, **kw)
    out = np.zeros((B, N1, C_S), np.float32)
    for c in range(NCORES):
        out[c // 4] += res.results[c]["out"]
    out += b_out[None, None, :]
    return out, res


def kernel(**inputs) -> np.ndarray:
    out, _ = _run(inputs, trace=False)
    return out


# revision 32
# speedup vs baseline: 1.1881x; 1.0581x over previous
"""Trainium2 Bass kernel for nn_Attention_12146167513140.

Distributed dense attention over 8 NeuronCores.

Sharding: core c in 0..7 -> (b = c//4, head-pair hp = c%4).  Each core
computes the full [3072 q x 3072 k] attention for its 2 heads of its
batch, producing a partial output projection [3072, 256]; the host sums
the 4 partials per batch and adds b_out.

Device pipeline per core (all matmuls bf16, accumulation f32 in PSUM):
  A) kv = s2 @ Wkv_pair -> rms-norm k -> kT tiles (PE transpose),
     v (+ones col) -> vx tiles
  B) q = s1e @ Wq_pair -> rms-norm q -> qT tiles
  C) flash-style: scoresT[k,q] = kT.T @ qT (33rd contraction row carries
     the additive mask as a rank-1 term), exp on ScalarE (scale fused),
     PV: oT[h] += vx.T @ expT (ones column accumulates the softmax
     denominator Z), normalize by 1/Z
  D) out_partial = oT.T @ Wout_pair

Host-side prep: sinusoidal positional embedding (index arithmetic),
transposes, bf16 casts, mask row encoding.
"""

import numpy as np
import ml_dtypes

import concourse.bacc as bacc
import concourse.mybir as mybir
from concourse import bass_utils
from concourse.tile import TileContext
from concourse.alu_op_type import AluOpType
from concourse.mybir import ActivationFunctionType as AF

AX = mybir.AxisListType
BF = mybir.dt.bfloat16
F32 = mybir.dt.float32
bf16 = ml_dtypes.bfloat16

B, N1, N2 = 2, 3072, 3072
C_S, H, D = 256, 8, 32
INF = 100000.0
EPS = 1e-8
SCALE = float(np.sqrt(1.0 / (3 * D)))

NCORES = 8
HPC = 2            # heads per core
KCH = N2 // 128    # 24 key chunks
QCH = N1 // 128    # 24 q row chunks
QB = 512           # q block for scores free dim
NQB = N1 // QB     # 6
VW = D + 1         # 33: v columns + ones column for Z

_cache = {}


def _build(use_g2: bool):
    nc = bacc.Bacc("TRN2", target_bir_lowering=False, debug=False, num_devices=NCORES)

    s1T_d = nc.dram_tensor("s1T", [C_S, N1], BF, kind="ExternalInput")
    s2T_d = nc.dram_tensor("s2T", [C_S, N2], BF, kind="ExternalInput")
    wq_d = nc.dram_tensor("wq", [C_S, HPC * D], BF, kind="ExternalInput")
    wkv_d = nc.dram_tensor("wkv", [C_S, HPC * 2 * D], BF, kind="ExternalInput")
    wout_d = nc.dram_tensor("wout", [HPC * D, C_S], BF, kind="ExternalInput")
    qm_d = nc.dram_tensor("qm", [1, N1], BF, kind="ExternalInput")
    km_d = nc.dram_tensor("km", [1, N2], BF, kind="ExternalInput")
    id_d = nc.dram_tensor("ident", [128, 128], BF, kind="ExternalInput")
    z_d = nc.dram_tensor("zeros", [96, N2], BF, kind="ExternalInput")
    if use_g2:
        g2_d = nc.dram_tensor("g2", [128, HPC * D], BF, kind="ExternalInput")
    zdr = [nc.dram_tensor(f"zscratch{h}", [1, N1], F32, kind="Internal")
           for h in range(HPC)]
    out_d = nc.dram_tensor("out", [N1, C_S], F32, kind="ExternalOutput")

    with TileContext(nc) as tc:
        with (
            tc.tile_pool(name="const", bufs=1) as cpool,
            tc.tile_pool(name="stage", bufs=1) as stage,
            tc.tile_pool(name="work", bufs=3) as work,
            tc.tile_pool(name="norm", bufs=3) as npool,
            tc.tile_pool(name="expp", bufs=3) as expp,
            tc.tile_pool(name="epi", bufs=4) as epi,
            tc.tile_pool(name="psA", bufs=2, space="PSUM") as psA,
            tc.tile_pool(name="psO", bufs=4, space="PSUM") as psO,
        ):
            # ---- constants / staging ----
            ident = cpool.tile([128, 128], BF)
            nc.sync.dma_start(ident[:, :], id_d.ap())
            epst = cpool.tile([128, 1], F32)
            nc.vector.memset(epst[:, :], EPS)

            wq_sb = cpool.tile([128, HPC * D], BF, tag="wq")
            wq_sb2 = cpool.tile([128, HPC * D], BF, tag="wq2")
            nc.sync.dma_start(wq_sb[:, :], wq_d.ap()[0:128, :])
            nc.sync.dma_start(wq_sb2[:, :], wq_d.ap()[128:256, :])
            wkv_sb = cpool.tile([128, HPC * 2 * D], BF, tag="wkv")
            wkv_sb2 = cpool.tile([128, HPC * 2 * D], BF, tag="wkv2")
            nc.sync.dma_start(wkv_sb[:, :], wkv_d.ap()[0:128, :])
            nc.sync.dma_start(wkv_sb2[:, :], wkv_d.ap()[128:256, :])
            wout_sb = cpool.tile([HPC * D, C_S], BF, tag="wout")
            nc.sync.dma_start(wout_sb[:, :], wout_d.ap())
            if use_g2:
                g2_sb = cpool.tile([128, HPC * D], BF, tag="g2")
                nc.sync.dma_start(g2_sb[:, :], g2_d.ap())

            s1T = [cpool.tile([128, N1], BF, tag=f"s1T{i}", name=f"s1T{i}") for i in range(2)]
            s2T = [cpool.tile([128, N2], BF, tag=f"s2T{i}", name=f"s2T{i}") for i in range(2)]
            for i in range(2):
                for j in range(4):
                    sl = slice(j * (N1 // 4), (j + 1) * (N1 // 4))
                    nc.sync.dma_start(s1T[i][:, sl], s1T_d.ap()[i * 128:(i + 1) * 128, sl])
                    nc.sync.dma_start(s2T[i][:, sl], s2T_d.ap()[i * 128:(i + 1) * 128, sl])

            # per-head transposed tensors; row 32 carries the mask row.
            # Padded to 128 partitions (rows 33..127 zero) so the QK matmul
            # streams full-width K=128 (K=33 runs ~1.7x slower).
            kT = [cpool.tile([128, N2], BF, tag=f"kT{h}", name=f"kT{h}") for h in range(HPC)]
            qT = [cpool.tile([128, N1], BF, tag=f"qT{h}", name=f"qT{h}") for h in range(HPC)]
            for h in range(HPC):
                nc.sync.dma_start(kT[h][32:128, :], z_d.ap())
                nc.sync.dma_start(qT[h][32:128, :], z_d.ap())
                nc.sync.dma_start(kT[h][32:33, :], km_d.ap())
                nc.sync.dma_start(qT[h][32:33, :], qm_d.ap())

            # v-extended: per kchunk, per head: [v(32) | ones(1)] columns
            vx = cpool.tile([128, KCH * HPC * VW], BF, tag="vx")
            nc.gpsimd.memset(
                vx[:, :].rearrange("p (n w) -> p n w", w=VW)[:, :, 32:33], 1.0
            )

            oT_sb = cpool.tile([HPC * D, N1], BF, tag="oT")
            zrow = [cpool.tile([1, N1], F32, tag=f"zrow{h}", name=f"zrow{h}")
                    for h in range(HPC)]
            zp = [cpool.tile([128, QCH], F32, tag=f"zp{h}", name=f"zp{h}")
                  for h in range(HPC)]
            rzp = [cpool.tile([128, QCH], F32, tag=f"rzp{h}", name=f"rzp{h}")
                   for h in range(HPC)]

            # ---- phases A/B: projections + rms-norm + transposes ----
            # Two passes with ONE batched Sqrt (avoids ACT table thrash
            # between the Sqrt and Exp function sets) and ONE batched
            # reciprocal (DVE reciprocal has a large per-op floor).
            NCHUNK = KCH + QCH  # 48
            kcp_all = cpool.tile([128, NCHUNK * HPC * D], F32, tag="kcp_all")
            ss_all = cpool.tile([128, NCHUNK * HPC], F32, tag="ss_all")

            def pass1(ci, kc, sT, w1, w2, vdst):
                ncol = w1.shape[1]
                pp = psO.tile([128, ncol], F32, tag="oT", name=f"pp{ci}")
                nc.tensor.matmul(pp[:, :], sT[0][:, kc * 128:(kc + 1) * 128], w1[:, :],
                                 start=True, stop=False)
                nc.tensor.matmul(pp[:, :], sT[1][:, kc * 128:(kc + 1) * 128], w2[:, :],
                                 start=False, stop=True)
                per_h = ncol // HPC
                kcp = kcp_all[:, ci * HPC * D:(ci + 1) * HPC * D]
                nc.vector.tensor_copy(
                    kcp.rearrange("p (h d) -> p h d", d=D),
                    pp[:, :].rearrange("p (h x) -> p h x", h=HPC)[:, :, 0:D])
                sq = npool.tile([128, HPC * D], F32, tag="sq", name=f"sq{ci}")
                nc.vector.tensor_tensor(sq[:, :], kcp, kcp, AluOpType.mult)
                nc.vector.reduce_sum(
                    ss_all[:, ci * HPC:(ci + 1) * HPC],
                    sq[:, :].rearrange("p (h d) -> p h d", d=D), axis=AX.X)
                if vdst is not None:  # kv: copy v columns into vx (+cast bf16)
                    nc.vector.tensor_copy(
                        vdst[:, kc * HPC * VW:(kc + 1) * HPC * VW]
                        .rearrange("p (h w) -> p h w", w=VW)[:, :, 0:D],
                        pp[:, :].rearrange("p (h x) -> p h x", h=HPC)[:, :, D:2 * D])

            sr_all = cpool.tile([128, NCHUNK * HPC], F32, tag="sr_all")
            rinv_all = cpool.tile([128, NCHUNK * HPC], F32, tag="rinv_all")

            def pass2(ci, kc, dstT, qside):
                kcp = kcp_all[:, ci * HPC * D:(ci + 1) * HPC * D]
                pre = npool.tile([128, HPC * D], BF, tag="pre", name=f"pre{ci}")
                for h in range(HPC):
                    nc.vector.tensor_scalar(
                        pre[:, h * D:(h + 1) * D], kcp[:, h * D:(h + 1) * D],
                        rinv_all[:, ci * HPC + h:ci * HPC + h + 1], None,
                        AluOpType.mult)
                if use_g2 and qside:  # q side carries the gq*gk factor
                    nc.vector.tensor_tensor(pre[:, :], pre[:, :], g2_sb[:, :],
                                            AluOpType.mult)
                tp = psO.tile([HPC * D, 128], BF, tag="oT", name=f"tp{ci}")
                nc.tensor.transpose(tp[:, :], pre[:, :], ident[:, :])
                for h in range(HPC):
                    nc.vector.tensor_copy(
                        dstT[h][0:D, kc * 128:(kc + 1) * 128],
                        tp[h * D:(h + 1) * D, :])

            def norm_batch(chunks, kvside):
                """pass1 for a contiguous chunk batch, one batched sqrt+recip,
                then pass2.  Sub-batching keeps the prologue pipelined: phase C
                unblocks as soon as the early q/kv batches land."""
                for kc in chunks:
                    if kvside:
                        pass1(kc, kc, s2T, wkv_sb, wkv_sb2, vx)
                    else:
                        pass1(KCH + kc, kc, s1T, wq_sb, wq_sb2, None)
                ci0 = (chunks[0] if kvside else KCH + chunks[0]) * HPC
                ci1 = (chunks[-1] if kvside else KCH + chunks[-1]) * HPC + HPC
                sl = slice(ci0, ci1)
                nc.scalar.activation(sr_all[:, sl], ss_all[:, sl], AF.Sqrt,
                                     bias=epst[:, :], scale=1.0 / D)
                nc.vector.reciprocal(rinv_all[:, sl], sr_all[:, sl])
                for kc in chunks:
                    if kvside:
                        pass2(kc, kc, kT, False)
                    else:
                        pass2(KCH + kc, kc, qT, True)

            def attend(qb, fillers=()):
                fillers = list(fillers)
                nf = len(fillers)
                qsl = slice(qb * QB, (qb + 1) * QB)
                oT = [psO.tile([VW, QB], F32, tag="oT", name=f"oT_{qb}_{i}")
                      for i in range(HPC)]
                for kc in range(KCH):
                    while fillers and (nf - len(fillers)) * KCH <= kc * nf:
                        fillers.pop(0)()
                    sc = psA.tile([128, HPC * QB], F32, tag="sc",
                                  name=f"sc_{qb}_{kc}")
                    for h in range(HPC):
                        nc.tensor.matmul(
                            sc[:, h * QB:(h + 1) * QB],
                            kT[h][:, kc * 128:(kc + 1) * 128],
                            qT[h][:, qsl],
                            start=True, stop=True)
                    ex = expp.tile([128, HPC * QB], BF, tag="ex",
                                   name=f"ex_{qb}_{kc}")
                    nc.scalar.activation(ex[:, :], sc[:, :], AF.Exp, scale=SCALE)
                    for h in range(HPC):
                        nc.tensor.matmul(
                            oT[h][:, :],
                            vx[:, (kc * HPC + h) * VW:(kc * HPC + h + 1) * VW],
                            ex[:, h * QB:(h + 1) * QB],
                            start=(kc == 0), stop=(kc == KCH - 1))
                for h in range(HPC):
                    # unnormalized o -> sbuf bf16; Z row -> per-head Z vector
                    nc.vector.tensor_copy(oT_sb[h * D:(h + 1) * D, qsl],
                                          oT[h][0:D, :])
                    nc.vector.tensor_copy(zrow[h][0:1, qsl], oT[h][32:33, :])
                # 1/Z in q-partition-major layout: rzp[h][p,qc] = 1/Z_h[qc*128+p]
                # via a DRAM bounce (partition<->free transpose), so the
                # reciprocal runs on [128, 4] instead of [1, 512].
                for h in range(HPC):
                    nc.sync.dma_start(zdr[h].ap()[0:1, qsl], zrow[h][0:1, qsl])
                    nc.sync.dma_start(
                        zp[h][:, qb * 4:(qb + 1) * 4],
                        zdr[h].ap()[0:1, qsl].rearrange("o (c p) -> o p c", p=128)[0])
                    nc.vector.reciprocal(rzp[h][:, qb * 4:(qb + 1) * 4],
                                         zp[h][:, qb * 4:(qb + 1) * 4])

            def proj_out(qc):
                osl = slice(qc * 128, (qc + 1) * 128)
                op0 = psO.tile([128, C_S], F32, tag="oT", name=f"op0_{qc}")
                nc.tensor.matmul(op0[:, :], oT_sb[0:D, osl], wout_sb[0:D, :],
                                 start=True, stop=True)
                op1 = psO.tile([128, C_S], F32, tag="oT", name=f"op1_{qc}")
                nc.tensor.matmul(op1[:, :], oT_sb[D:2 * D, osl],
                                 wout_sb[D:2 * D, :], start=True, stop=True)
                t0 = work.tile([128, C_S], F32, tag="t0", name=f"t0_{qc}")
                nc.vector.tensor_scalar(t0[:, :], op0[:, :],
                                        rzp[0][:, qc:qc + 1], None,
                                        AluOpType.mult)
                ops = work.tile([128, C_S], F32, tag="osb", name=f"osb_{qc}")
                nc.vector.scalar_tensor_tensor(
                    ops[:, :], op1[:, :], rzp[1][:, qc:qc + 1], t0[:, :],
                    AluOpType.mult, AluOpType.add)
                nc.sync.dma_start(out_d.ap()[osl, :], ops[:, :])

            # prologue schedule: attend(0) needs qT chunks 0-3 and kT
            # incrementally; emit the minimum before it and overlap the rest.
            # The out-projection for q-block qb is emitted after attend(qb+1)
            # so its matmuls fill PE bubbles instead of forming a tail.
            norm_batch(list(range(0, 8)), kvside=False)
            norm_batch(list(range(0, 12)), kvside=True)
            norm_batch(list(range(12, 24)), kvside=True)
            # q chunks 8-23 are not needed until attend(2): run their pass1
            # as fillers inside attend(0)'s kc loop and pass2 inside
            # attend(1)'s, so their matmuls/DVE work fill pipeline bubbles
            # instead of blocking the attention stream in program order.
            f0 = [(lambda kc=kc: pass1(KCH + kc, kc, s1T, wq_sb, wq_sb2, None))
                  for kc in range(8, 24)]
            attend(0, f0)
            q_sl2 = slice((KCH + 8) * HPC, (KCH + 24) * HPC)
            nc.scalar.activation(sr_all[:, q_sl2], ss_all[:, q_sl2], AF.Sqrt,
                                 bias=epst[:, :], scale=1.0 / D)
            nc.vector.reciprocal(rinv_all[:, q_sl2], sr_all[:, q_sl2])
            f1 = [(lambda kc=kc: pass2(KCH + kc, kc, qT, True))
                  for kc in range(8, 24)]
            attend(1, f1)
            for qb in range(2, NQB):
                fd = [(lambda qc=qc: proj_out(qc))
                      for qc in range((qb - 2) * 4, (qb - 1) * 4)]
                attend(qb, fd)
            for qc in range((NQB - 2) * 4, NQB * 4):
                proj_out(qc)

    nc.compile()
    return nc


def _host_prep(inputs):
    s1 = np.asarray(inputs["s1"], np.float32)
    s2 = np.asarray(inputs["s2"], np.float32)
    ridx1 = np.asarray(inputs["ridx1"], np.int32)
    ct1 = np.asarray(inputs["ct1"], np.int32)
    mask1 = np.asarray(inputs["mask1"], np.int32)
    mask2 = np.asarray(inputs["mask2"], np.int32)
    Wq = np.asarray(inputs["Wq"], np.float32)
    Wkv = np.asarray(inputs["Wkv"], np.float32)
    Wout = np.asarray(inputs["Wout"], np.float32)
    gq = np.asarray(inputs["gq"], np.float32)
    gk = np.asarray(inputs["gk"], np.float32)

    ct_idx = np.take_along_axis(ridx1, ct1[:, None], axis=1)
    pos = (ridx1 - ct_idx).astype(np.float32)
    half = C_S // 2
    freqs = np.exp(-np.log(10000.0) * np.arange(half, dtype=np.float32) / half)
    ang = pos[..., None] * freqs
    s1e = s1 + np.concatenate([np.sin(ang), np.cos(ang)], axis=-1).astype(np.float32)

    m1 = mask1.astype(np.float32)
    km = (mask2.astype(np.float32) - 1.0) * INF / SCALE

    g2 = gq * gk
    use_g2 = not np.allclose(g2, 1.0)

    ident = np.eye(128, dtype=bf16)
    zeros = np.zeros((96, N2), dtype=bf16)
    in_maps = []
    for c in range(NCORES):
        b, hp = c // 4, c % 4
        m = {
            "s1T": np.ascontiguousarray(s1e[b].T).astype(bf16),
            "s2T": np.ascontiguousarray(s2[b].T).astype(bf16),
            "wq": np.ascontiguousarray(Wq[:, hp * HPC * D:(hp + 1) * HPC * D]).astype(bf16),
            "wkv": np.ascontiguousarray(Wkv[:, hp * HPC * 2 * D:(hp + 1) * HPC * 2 * D]).astype(bf16),
            "wout": np.ascontiguousarray(Wout[hp * HPC * D:(hp + 1) * HPC * D, :]).astype(bf16),
            "qm": m1[b][None, :].astype(bf16),
            "km": km[b][None, :].astype(bf16),
            "ident": ident,
            "zeros": zeros,
        }
        if use_g2:
            m["g2"] = np.tile(g2[None, hp * HPC * D:(hp + 1) * HPC * D], (128, 1)).astype(bf16)
        in_maps.append(m)
    return in_maps, use_g2, np.asarray(inputs["b_out"], np.float32)


def _run(inputs, trace=False, **kw):
    in_maps, use_g2, b_out = _host_prep(inputs)
    key = ("nc", use_g2)
    if key not in _cache:
        _cache[key] = _build(use_g2)
    nc = _cache[key]
    res = bass_utils.run_bass_kernel_spmd(
        nc, in_maps, core_ids=list(range(NCORES)), trace=trace, **kw)
    out = np.zeros((B, N1, C_S), np.float32)
    for c in range(NCORES):
        out[c // 4] += res.results[c]["out"]
    out += b_out[None, None, :]
    return out, res


def kernel(**inputs) -> np.ndarray:
    out, _ = _run(inputs, trace=False)
    return out


# revision 33
# speedup vs baseline: 1.1968x; 1.0073x over previous
"""Trainium2 Bass kernel for nn_Attention_12146167513140.

Distributed dense attention over 8 NeuronCores.

Sharding: core c in 0..7 -> (b = c//4, head-pair hp = c%4).  Each core
computes the full [3072 q x 3072 k] attention for its 2 heads of its
batch, producing a partial output projection [3072, 256]; the host sums
the 4 partials per batch and adds b_out.

Device pipeline per core (all matmuls bf16, accumulation f32 in PSUM):
  A) kv = s2 @ Wkv_pair -> rms-norm k -> kT tiles (PE transpose),
     v (+ones col) -> vx tiles
  B) q = s1e @ Wq_pair -> rms-norm q -> qT tiles
  C) flash-style: scoresT[k,q] = kT.T @ qT (33rd contraction row carries
     the additive mask as a rank-1 term), exp on ScalarE (scale fused),
     PV: oT[h] += vx.T @ expT (ones column accumulates the softmax
     denominator Z), normalize by 1/Z
  D) out_partial = oT.T @ Wout_pair

Host-side prep: sinusoidal positional embedding (index arithmetic),
transposes, bf16 casts, mask row encoding.
"""

import numpy as np
import ml_dtypes

import concourse.bacc as bacc
import concourse.mybir as mybir
from concourse import bass_utils
from concourse.tile import TileContext
from concourse.alu_op_type import AluOpType
from concourse.mybir import ActivationFunctionType as AF

AX = mybir.AxisListType
BF = mybir.dt.bfloat16
F32 = mybir.dt.float32
bf16 = ml_dtypes.bfloat16

B, N1, N2 = 2, 3072, 3072
C_S, H, D = 256, 8, 32
INF = 100000.0
EPS = 1e-8
SCALE = float(np.sqrt(1.0 / (3 * D)))

NCORES = 8
HPC = 2            # heads per core
KCH = N2 // 128    # 24 key chunks
QCH = N1 // 128    # 24 q row chunks
QB = 512           # q block for scores free dim
NQB = N1 // QB     # 6
VW = D + 1         # 33: v columns + ones column for Z

_cache = {}


def _build(use_g2: bool):
    nc = bacc.Bacc("TRN2", target_bir_lowering=False, debug=False, num_devices=NCORES)

    s1T_d = nc.dram_tensor("s1T", [C_S, N1], BF, kind="ExternalInput")
    s2T_d = nc.dram_tensor("s2T", [C_S, N2], BF, kind="ExternalInput")
    wq_d = nc.dram_tensor("wq", [C_S, HPC * D], BF, kind="ExternalInput")
    wkv_d = nc.dram_tensor("wkv", [C_S, HPC * 2 * D], BF, kind="ExternalInput")
    wout_d = nc.dram_tensor("wout", [HPC * D, C_S], BF, kind="ExternalInput")
    qm_d = nc.dram_tensor("qm", [1, N1], BF, kind="ExternalInput")
    km_d = nc.dram_tensor("km", [1, N2], BF, kind="ExternalInput")
    id_d = nc.dram_tensor("ident", [128, 128], BF, kind="ExternalInput")
    z_d = nc.dram_tensor("zeros", [96, N2], BF, kind="ExternalInput")
    if use_g2:
        g2_d = nc.dram_tensor("g2", [128, HPC * D], BF, kind="ExternalInput")
    zdr = [nc.dram_tensor(f"zscratch{h}", [1, N1], F32, kind="Internal")
           for h in range(HPC)]
    out_d = nc.dram_tensor("out", [N1, C_S], F32, kind="ExternalOutput")

    with TileContext(nc) as tc:
        with (
            tc.tile_pool(name="const", bufs=1) as cpool,
            tc.tile_pool(name="stage", bufs=1) as stage,
            tc.tile_pool(name="work", bufs=3) as work,
            tc.tile_pool(name="norm", bufs=6) as npool,
            tc.tile_pool(name="expp", bufs=3) as expp,
            tc.tile_pool(name="epi", bufs=4) as epi,
            tc.tile_pool(name="psA", bufs=2, space="PSUM") as psA,
            tc.tile_pool(name="psO", bufs=4, space="PSUM") as psO,
        ):
            # ---- constants / staging ----
            ident = cpool.tile([128, 128], BF)
            nc.sync.dma_start(ident[:, :], id_d.ap())
            epst = cpool.tile([128, 1], F32)
            nc.vector.memset(epst[:, :], EPS)

            wq_sb = cpool.tile([128, HPC * D], BF, tag="wq")
            wq_sb2 = cpool.tile([128, HPC * D], BF, tag="wq2")
            nc.sync.dma_start(wq_sb[:, :], wq_d.ap()[0:128, :])
            nc.sync.dma_start(wq_sb2[:, :], wq_d.ap()[128:256, :])
            wkv_sb = cpool.tile([128, HPC * 2 * D], BF, tag="wkv")
            wkv_sb2 = cpool.tile([128, HPC * 2 * D], BF, tag="wkv2")
            nc.sync.dma_start(wkv_sb[:, :], wkv_d.ap()[0:128, :])
            nc.sync.dma_start(wkv_sb2[:, :], wkv_d.ap()[128:256, :])
            wout_sb = cpool.tile([HPC * D, C_S], BF, tag="wout")
            nc.sync.dma_start(wout_sb[:, :], wout_d.ap())
            if use_g2:
                g2_sb = cpool.tile([128, HPC * D], BF, tag="g2")
                nc.sync.dma_start(g2_sb[:, :], g2_d.ap())

            s1T = [cpool.tile([128, N1], BF, tag=f"s1T{i}", name=f"s1T{i}") for i in range(2)]
            s2T = [cpool.tile([128, N2], BF, tag=f"s2T{i}", name=f"s2T{i}") for i in range(2)]
            for i in range(2):
                for j in range(4):
                    sl = slice(j * (N1 // 4), (j + 1) * (N1 // 4))
                    nc.sync.dma_start(s1T[i][:, sl], s1T_d.ap()[i * 128:(i + 1) * 128, sl])
                    nc.sync.dma_start(s2T[i][:, sl], s2T_d.ap()[i * 128:(i + 1) * 128, sl])

            # per-head transposed tensors; row 32 carries the mask row.
            # Padded to 128 partitions (rows 33..127 zero) so the QK matmul
            # streams full-width K=128 (K=33 runs ~1.7x slower).
            kT = [cpool.tile([128, N2], BF, tag=f"kT{h}", name=f"kT{h}") for h in range(HPC)]
            qT = [cpool.tile([128, N1], BF, tag=f"qT{h}", name=f"qT{h}") for h in range(HPC)]
            for h in range(HPC):
                nc.sync.dma_start(kT[h][32:128, :], z_d.ap())
                nc.sync.dma_start(qT[h][32:128, :], z_d.ap())
                nc.sync.dma_start(kT[h][32:33, :], km_d.ap())
                nc.sync.dma_start(qT[h][32:33, :], qm_d.ap())

            # v-extended: per kchunk, per head: [v(32) | ones(1)] columns
            vx = cpool.tile([128, KCH * HPC * VW], BF, tag="vx")
            nc.gpsimd.memset(
                vx[:, :].rearrange("p (n w) -> p n w", w=VW)[:, :, 32:33], 1.0
            )

            oT_sb = cpool.tile([HPC * D, N1], BF, tag="oT")
            zrow = [cpool.tile([1, N1], F32, tag=f"zrow{h}", name=f"zrow{h}")
                    for h in range(HPC)]
            zp = [cpool.tile([128, QCH], F32, tag=f"zp{h}", name=f"zp{h}")
                  for h in range(HPC)]
            rzp = [cpool.tile([128, QCH], F32, tag=f"rzp{h}", name=f"rzp{h}")
                   for h in range(HPC)]

            # ---- phases A/B: projections + rms-norm + transposes ----
            # Two passes with ONE batched Sqrt (avoids ACT table thrash
            # between the Sqrt and Exp function sets) and ONE batched
            # reciprocal (DVE reciprocal has a large per-op floor).
            NCHUNK = KCH + QCH  # 48
            kcp_all = cpool.tile([128, NCHUNK * HPC * D], F32, tag="kcp_all")
            ss_all = cpool.tile([128, NCHUNK * HPC], F32, tag="ss_all")

            def pass1(ci, kc, sT, w1, w2, vdst):
                ncol = w1.shape[1]
                pp = psO.tile([128, ncol], F32, tag="oT", name=f"pp{ci}")
                nc.tensor.matmul(pp[:, :], sT[0][:, kc * 128:(kc + 1) * 128], w1[:, :],
                                 start=True, stop=False)
                nc.tensor.matmul(pp[:, :], sT[1][:, kc * 128:(kc + 1) * 128], w2[:, :],
                                 start=False, stop=True)
                per_h = ncol // HPC
                kcp = kcp_all[:, ci * HPC * D:(ci + 1) * HPC * D]
                nc.scalar.copy(
                    kcp.rearrange("p (h d) -> p h d", d=D),
                    pp[:, :].rearrange("p (h x) -> p h x", h=HPC)[:, :, 0:D])
                sq = npool.tile([128, HPC * D], F32, tag="sq", name=f"sq{ci}")
                nc.vector.tensor_tensor(sq[:, :], kcp, kcp, AluOpType.mult)
                nc.vector.reduce_sum(
                    ss_all[:, ci * HPC:(ci + 1) * HPC],
                    sq[:, :].rearrange("p (h d) -> p h d", d=D), axis=AX.X)
                if vdst is not None:  # kv: copy v columns into vx (+cast bf16)
                    nc.scalar.copy(
                        vdst[:, kc * HPC * VW:(kc + 1) * HPC * VW]
                        .rearrange("p (h w) -> p h w", w=VW)[:, :, 0:D],
                        pp[:, :].rearrange("p (h x) -> p h x", h=HPC)[:, :, D:2 * D])

            sr_all = cpool.tile([128, NCHUNK * HPC], F32, tag="sr_all")
            rinv_all = cpool.tile([128, NCHUNK * HPC], F32, tag="rinv_all")

            def pass2(ci, kc, dstT, qside):
                kcp = kcp_all[:, ci * HPC * D:(ci + 1) * HPC * D]
                pre = npool.tile([128, HPC * D], BF, tag="pre", name=f"pre{ci}")
                for h in range(HPC):
                    nc.vector.tensor_scalar(
                        pre[:, h * D:(h + 1) * D], kcp[:, h * D:(h + 1) * D],
                        rinv_all[:, ci * HPC + h:ci * HPC + h + 1], None,
                        AluOpType.mult)
                if use_g2 and qside:  # q side carries the gq*gk factor
                    nc.vector.tensor_tensor(pre[:, :], pre[:, :], g2_sb[:, :],
                                            AluOpType.mult)
                tp = psO.tile([HPC * D, 128], BF, tag="oT", name=f"tp{ci}")
                nc.tensor.transpose(tp[:, :], pre[:, :], ident[:, :])
                for h in range(HPC):
                    nc.vector.tensor_copy(
                        dstT[h][0:D, kc * 128:(kc + 1) * 128],
                        tp[h * D:(h + 1) * D, :])

            def norm_batch(chunks, kvside):
                """pass1 for a contiguous chunk batch, one batched sqrt+recip,
                then pass2.  Sub-batching keeps the prologue pipelined: phase C
                unblocks as soon as the early q/kv batches land."""
                for kc in chunks:
                    if kvside:
                        pass1(kc, kc, s2T, wkv_sb, wkv_sb2, vx)
                    else:
                        pass1(KCH + kc, kc, s1T, wq_sb, wq_sb2, None)
                ci0 = (chunks[0] if kvside else KCH + chunks[0]) * HPC
                ci1 = (chunks[-1] if kvside else KCH + chunks[-1]) * HPC + HPC
                sl = slice(ci0, ci1)
                nc.scalar.activation(sr_all[:, sl], ss_all[:, sl], AF.Sqrt,
                                     bias=epst[:, :], scale=1.0 / D)
                nc.vector.reciprocal(rinv_all[:, sl], sr_all[:, sl])
                for kc in chunks:
                    if kvside:
                        pass2(kc, kc, kT, False)
                    else:
                        pass2(KCH + kc, kc, qT, True)

            def attend(qb, fillers=()):
                fillers = list(fillers)
                nf = len(fillers)
                qsl = slice(qb * QB, (qb + 1) * QB)
                oT = [psO.tile([VW, QB], F32, tag="oT", name=f"oT_{qb}_{i}")
                      for i in range(HPC)]
                for kc in range(KCH):
                    while fillers and (nf - len(fillers)) * KCH <= kc * nf:
                        fillers.pop(0)()
                    sc = psA.tile([128, HPC * QB], F32, tag="sc",
                                  name=f"sc_{qb}_{kc}")
                    for h in range(HPC):
                        nc.tensor.matmul(
                            sc[:, h * QB:(h + 1) * QB],
                            kT[h][:, kc * 128:(kc + 1) * 128],
                            qT[h][:, qsl],
                            start=True, stop=True)
                    ex = expp.tile([128, HPC * QB], BF, tag="ex",
                                   name=f"ex_{qb}_{kc}")
                    nc.scalar.activation(ex[:, :], sc[:, :], AF.Exp, scale=SCALE)
                    for h in range(HPC):
                        nc.tensor.matmul(
                            oT[h][:, :],
                            vx[:, (kc * HPC + h) * VW:(kc * HPC + h + 1) * VW],
                            ex[:, h * QB:(h + 1) * QB],
                            start=(kc == 0), stop=(kc == KCH - 1))
                for h in range(HPC):
                    # unnormalized o -> sbuf bf16; Z row -> per-head Z vector
                    nc.vector.tensor_copy(oT_sb[h * D:(h + 1) * D, qsl],
                                          oT[h][0:D, :])
                    nc.vector.tensor_copy(zrow[h][0:1, qsl], oT[h][32:33, :])
                # 1/Z in q-partition-major layout: rzp[h][p,qc] = 1/Z_h[qc*128+p]
                # via a DRAM bounce (partition<->free transpose), so the
                # reciprocal runs on [128, 4] instead of [1, 512].
                for h in range(HPC):
                    nc.sync.dma_start(zdr[h].ap()[0:1, qsl], zrow[h][0:1, qsl])
                    nc.sync.dma_start(
                        zp[h][:, qb * 4:(qb + 1) * 4],
                        zdr[h].ap()[0:1, qsl].rearrange("o (c p) -> o p c", p=128)[0])
                    nc.vector.reciprocal(rzp[h][:, qb * 4:(qb + 1) * 4],
                                         zp[h][:, qb * 4:(qb + 1) * 4])

            def proj_out(qc):
                osl = slice(qc * 128, (qc + 1) * 128)
                op0 = psO.tile([128, C_S], F32, tag="oT", name=f"op0_{qc}")
                nc.tensor.matmul(op0[:, :], oT_sb[0:D, osl], wout_sb[0:D, :],
                                 start=True, stop=True)
                op1 = psO.tile([128, C_S], F32, tag="oT", name=f"op1_{qc}")
                nc.tensor.matmul(op1[:, :], oT_sb[D:2 * D, osl],
                                 wout_sb[D:2 * D, :], start=True, stop=True)
                t0 = work.tile([128, C_S], F32, tag="t0", name=f"t0_{qc}")
                nc.vector.tensor_scalar(t0[:, :], op0[:, :],
                                        rzp[0][:, qc:qc + 1], None,
                                        AluOpType.mult)
                ops = work.tile([128, C_S], F32, tag="osb", name=f"osb_{qc}")
                nc.vector.scalar_tensor_tensor(
                    ops[:, :], op1[:, :], rzp[1][:, qc:qc + 1], t0[:, :],
                    AluOpType.mult, AluOpType.add)
                nc.sync.dma_start(out_d.ap()[osl, :], ops[:, :])

            # prologue schedule: attend(0) needs qT chunks 0-3 and kT
            # incrementally; emit the minimum before it and overlap the rest.
            # The out-projection for q-block qb is emitted after attend(qb+1)
            # so its matmuls fill PE bubbles instead of forming a tail.
            norm_batch(list(range(0, 8)), kvside=False)
            norm_batch(list(range(0, 12)), kvside=True)
            norm_batch(list(range(12, 24)), kvside=True)
            # q chunks 8-23 are not needed until attend(2): run their pass1
            # as fillers inside attend(0)'s kc loop and pass2 inside
            # attend(1)'s, so their matmuls/DVE work fill pipeline bubbles
            # instead of blocking the attention stream in program order.
            f0 = [(lambda kc=kc: pass1(KCH + kc, kc, s1T, wq_sb, wq_sb2, None))
                  for kc in range(8, 24)]
            attend(0, f0)
            q_sl2 = slice((KCH + 8) * HPC, (KCH + 24) * HPC)
            nc.scalar.activation(sr_all[:, q_sl2], ss_all[:, q_sl2], AF.Sqrt,
                                 bias=epst[:, :], scale=1.0 / D)
            nc.vector.reciprocal(rinv_all[:, q_sl2], sr_all[:, q_sl2])
            f1 = [(lambda kc=kc: pass2(KCH + kc, kc, qT, True))
                  for kc in range(8, 24)]
            attend(1, f1)
            for qb in range(2, NQB):
                fd = [(lambda qc=qc: proj_out(qc))
                      for qc in range((qb - 2) * 4, (qb - 1) * 4)]
                attend(qb, fd)
            for qc in range((NQB - 2) * 4, NQB * 4):
                proj_out(qc)

    nc.compile()
    return nc


def _host_prep(inputs):
    s1 = np.asarray(inputs["s1"], np.float32)
    s2 = np.asarray(inputs["s2"], np.float32)
    ridx1 = np.asarray(inputs["ridx1"], np.int32)
    ct1 = np.asarray(inputs["ct1"], np.int32)
    mask1 = np.asarray(inputs["mask1"], np.int32)
    mask2 = np.asarray(inputs["mask2"], np.int32)
    Wq = np.asarray(inputs["Wq"], np.float32)
    Wkv = np.asarray(inputs["Wkv"], np.float32)
    Wout = np.asarray(inputs["Wout"], np.float32)
    gq = np.asarray(inputs["gq"], np.float32)
    gk = np.asarray(inputs["gk"], np.float32)

    ct_idx = np.take_along_axis(ridx1, ct1[:, None], axis=1)
    pos = (ridx1 - ct_idx).astype(np.float32)
    half = C_S // 2
    freqs = np.exp(-np.log(10000.0) * np.arange(half, dtype=np.float32) / half)
    ang = pos[..., None] * freqs
    s1e = s1 + np.concatenate([np.sin(ang), np.cos(ang)], axis=-1).astype(np.float32)

    m1 = mask1.astype(np.float32)
    km = (mask2.astype(np.float32) - 1.0) * INF / SCALE

    g2 = gq * gk
    use_g2 = not np.allclose(g2, 1.0)

    ident = np.eye(128, dtype=bf16)
    zeros = np.zeros((96, N2), dtype=bf16)
    in_maps = []
    for c in range(NCORES):
        b, hp = c // 4, c % 4
        m = {
            "s1T": np.ascontiguousarray(s1e[b].T).astype(bf16),
            "s2T": np.ascontiguousarray(s2[b].T).astype(bf16),
            "wq": np.ascontiguousarray(Wq[:, hp * HPC * D:(hp + 1) * HPC * D]).astype(bf16),
            "wkv": np.ascontiguousarray(Wkv[:, hp * HPC * 2 * D:(hp + 1) * HPC * 2 * D]).astype(bf16),
            "wout": np.ascontiguousarray(Wout[hp * HPC * D:(hp + 1) * HPC * D, :]).astype(bf16),
            "qm": m1[b][None, :].astype(bf16),
            "km": km[b][None, :].astype(bf16),
            "ident": ident,
            "zeros": zeros,
        }
        if use_g2:
            m["g2"] = np.tile(g2[None, hp * HPC * D:(hp + 1) * HPC * D], (128, 1)).astype(bf16)
        in_maps.append(m)
    return in_maps, use_g2, np.asarray(inputs["b_out"], np.float32)


def _run(inputs, trace=False, **kw):
    in_maps, use_g2, b_out = _host_prep(inputs)
    key = ("nc", use_g2)
    if key not in _cache:
        _cache[key] = _build(use_g2)
    nc = _cache[key]
    res = bass_utils.run_bass_kernel_spmd(
        nc, in_maps, core_ids=list(range(NCORES)), trace=trace, **kw)
    out = np.zeros((B, N1, C_S), np.float32)
    for c in range(NCORES):
        out[c // 4] += res.results[c]["out"]
    out += b_out[None, None, :]
    return out, res


def kernel(**inputs) -> np.ndarray:
    out, _ = _run(inputs, trace=False)
    return out
